# revision 5
# baseline (speedup 1.0000x reference)
"""Batched ragged segment-mean (BERTEmbedder merge loop) on 8 TRN2 NeuronCores.

Strategy
--------
Data-parallel over the batch: core c gets sequences [2c, 2c+1].  Within a
sequence, segment-sum is computed as a block-sparse one-hot matmul on the PE:

    out[t, d] = sum_s onehot[s, t] * x[s, d]

Segment ids are sorted per row, so each 128-subtoken tile only covers a
narrow window of token ids.  The host inspects the ids and builds a static
(s_tile, t_tile) pair schedule: for each 128-wide s-tile we emit matmuls only
into the 128-row t-tiles its ids can touch (union over the sequences that
share the SPMD program slot, so one program serves all 8 cores).  A column of
ones appended to the moving operand accumulates per-token counts in the same
PSUM tile; a reciprocal-multiply then turns sums into means.

The one-hot stationary operand is built on the DVE from a static iota and the
per-partition segment id: onehot[p, f] = (iota[f] + 128*j == sid[p]).  The
segment ids reach the partition dimension via a K=1 matmul against a column
of ones (a cheap PE "transpose" of 128 ids at a time).
"""

import os
import numpy as np

B, S, D, T, P = 16, 4096, 768, 2048, 128
NCORES = 8
SPC = B // NCORES          # sequences per core
NST, NTT = S // P, T // P  # 32 s-tiles, 16 t-tiles
DSPLIT = 512               # fp32 matmul moving-operand max
SUPER = 4                  # s-tiles per x-load DMA

_cache: dict = {}


def _schedule(segment_ids: np.ndarray):
    """Per program slot q: which t-tiles each s-tile touches, unioned over the
    sequences that run in that slot on every core (SPMD: one program)."""
    sched = []
    for q in range(SPC):
        seqs = [c * SPC + q for c in range(NCORES)]
        js_of = []
        for i in range(NST):
            blk = segment_ids[seqs, i * P:(i + 1) * P]
            lo, hi = int(blk.min()), int(blk.max())
            js_of.append(list(range(lo // P, hi // P + 1)))
        first, last = {}, {}
        for i in range(NST):
            for j in js_of[i]:
                first.setdefault(j, i)
                last[j] = i
        sched.append((tuple(tuple(js) for js in js_of),
                      tuple(sorted(first.items())),
                      tuple(sorted(last.items()))))
    return tuple(sched)


def _build(sched):
    from contextlib import ExitStack
    import concourse.bacc as bacc
    import concourse.tile as tile
    import concourse.mybir as mybir

    f32, i32 = mybir.dt.float32, mybir.dt.int32
    nc = bacc.Bacc("TRN2", target_bir_lowering=False, debug=False)
    x = nc.dram_tensor("raw_output", [SPC, S, D], f32, kind="ExternalInput").ap()
    sid = nc.dram_tensor("segment_ids", [SPC, S], i32, kind="ExternalInput").ap()
    out = nc.dram_tensor("out", [SPC, T, D], f32, kind="ExternalOutput").ap()

    with ExitStack() as ctx:
        tc = ctx.enter_context(tile.TileContext(nc))
        const = ctx.enter_context(tc.tile_pool(name="const", bufs=1))
        xp = ctx.enter_context(tc.tile_pool(name="xp", bufs=3))
        ohp = ctx.enter_context(tc.tile_pool(name="ohp", bufs=8))
        outp = ctx.enter_context(tc.tile_pool(name="outp", bufs=4))
        smp = ctx.enter_context(tc.tile_pool(name="smp", bufs=6))
        sidp = ctx.enter_context(tc.tile_pool(name="sidp", bufs=2))
        psb = ctx.enter_context(tc.tile_pool(name="psb", bufs=3, space="PSUM"))
        pss = ctx.enter_context(tc.tile_pool(name="pss", bufs=2, space="PSUM"))

        iota_i = const.tile([P, P], i32)
        nc.gpsimd.iota(iota_i[:], pattern=[[1, P]], base=0, channel_multiplier=0)
        iota_f = const.tile([P, P], f32)
        nc.vector.tensor_copy(iota_f[:], iota_i[:])
        ones = const.tile([1, 1], f32)
        nc.vector.memset(ones[:], 1.0)

        for q in range(SPC):
            js_of, first_t, last_t = sched[q]
            first = dict(first_t)
            last = dict(last_t)
            sid_row = sidp.tile([1, S], f32)
            nc.gpsimd.dma_start(out=sid_row[:], in_=sid[q][None, :])  # i32 -> f32 cast
            x_seq = x[q].rearrange("(n p) d -> p n d", p=P)  # [128, 32, 768]
            open_ps = {}
            for g in range(NST // SUPER):
                xt = xp.tile([P, SUPER, D + 1], f32)
                nc.sync.dma_start(out=xt[:, :, 0:D],
                                  in_=x_seq[:, g * SUPER:(g + 1) * SUPER, :])
                nc.vector.memset(xt[:, :, D:D + 1], 1.0)
                for si in range(SUPER):
                    i = g * SUPER + si
                    # segment ids of this s-tile onto the partition dim
                    sp_ps = pss.tile([P, 1], f32)
                    nc.tensor.matmul(sp_ps[:], lhsT=sid_row[0:1, i * P:(i + 1) * P],
                                     rhs=ones[:], start=True, stop=True)
                    sid_pt = smp.tile([P, 1], f32, tag="sidpt")
                    nc.vector.tensor_copy(sid_pt[:], sp_ps[:])
                    for j in js_of[i]:
                        oh = ohp.tile([P, P], f32)
                        # oh[p, f] = (iota[f] + 128*j == sid[p])
                        nc.vector.tensor_scalar(
                            oh[:], iota_f[:], float(j * P), sid_pt[:],
                            mybir.AluOpType.add, mybir.AluOpType.is_equal)
                        st, sp_ = (first[j] == i), (last[j] == i)
                        if st:
                            open_ps[j] = psb.tile([P, D + 1], f32, tag="acc",
                                                  name=f"acc_q{q}_j{j}")
                        pj = open_ps[j]
                        nc.tensor.matmul(pj[:, 0:DSPLIT], lhsT=oh[:],
                                         rhs=xt[:, si, 0:DSPLIT], start=st, stop=sp_)
                        nc.tensor.matmul(pj[:, DSPLIT:D + 1], lhsT=oh[:],
                                         rhs=xt[:, si, DSPLIT:D + 1], start=st, stop=sp_)
                        if sp_:
                            cnt = smp.tile([P, 1], f32, tag="cnt")
                            nc.vector.tensor_scalar_max(cnt[:], pj[:, D:D + 1], 1.0)
                            rec = smp.tile([P, 1], f32, tag="rec")
                            nc.vector.reciprocal(rec[:], cnt[:])
                            ot = outp.tile([P, D], f32)
                            nc.vector.tensor_scalar_mul(ot[:], pj[:, 0:D], rec[:])
                            nc.sync.dma_start(out=out[q, j * P:(j + 1) * P, :], in_=ot[:])
                            del open_ps[j]
            # t-tiles no s-tile can touch: all-empty segments -> zeros
            for j in range(NTT):
                if j not in first:
                    zt = outp.tile([P, D], f32)
                    nc.vector.memset(zt[:], 0.0)
                    nc.sync.dma_start(out=out[q, j * P:(j + 1) * P, :], in_=zt[:])
    nc.compile()
    return nc


def _get_nc(segment_ids: np.ndarray):
    sched = _schedule(segment_ids)
    if sched not in _cache:
        _cache[sched] = _build(sched)
    return _cache[sched]


def run(raw_output, segment_ids, trace=False):
    from concourse.bass_utils import run_bass_kernel_spmd

    raw_output = np.ascontiguousarray(raw_output, dtype=np.float32)
    segment_ids = np.ascontiguousarray(segment_ids, dtype=np.int32)
    nc = _get_nc(segment_ids)
    in_maps = [{"raw_output": raw_output[c * SPC:(c + 1) * SPC],
                "segment_ids": segment_ids[c * SPC:(c + 1) * SPC]}
               for c in range(NCORES)]
    bkr = run_bass_kernel_spmd(nc, in_maps, list(range(NCORES)), trace=trace)
    full = np.concatenate([bkr.results[c]["out"] for c in range(NCORES)], axis=0)
    return full, bkr


def kernel(raw_output, segment_ids):
    full, _ = run(raw_output, segment_ids,
                  trace=bool(int(os.environ.get("KERNEL_TRACE", "0"))))
    return full
